# revision 11
# baseline (speedup 1.0000x reference)
"""Trainium2 Bass kernel for nn_CausalSelfAttentionSynapse.

Math (per reference):
    qk = g @ W_lift.T                       # (B,T,2E)
    q,k heads of dim D=64; scores = q@k.T causal-masked
    lse[b,h,t] = logsumexp_{j<=t} scores[b,h,t,j]
    out[b,t]  = sum_h lse[b,h,t] * w[h],  w[h] = sum_g W_proj[g,h]

Sharding: 8 cores = 4 batches x 2 head-groups (8 heads each).

Per-core design (v4):
  - Host pre-transposes g[b] / W_lift head-group rows into e-major bf16
    (no device transposes, half DMA). g in 4 t-quarter tiles, W in 4
    head-pair tiles so lift starts ~5us in.
  - Lift (bf16, e on partitions) lands qkT per pair in score layout:
    partitions 0-63 head A d, 64-127 head B d; per t-quarter tile
    cols 0:512 q, 512:1024 k.
  - Scores: 64-deep matmuls; paired units alternate base partitions
    0/64 so the PE row-tiles two units at once. Causal mask of each
    diagonal block = identity x strict-upper(-30000) matmul; within a
    PSUM bank all writers form one contiguous accumulation group.
  - exp+row-sum is split across BOTH ScalarE and VectorE to balance:
      * ACT units: ScalarE Exp with fused accum_out (one per (h,qi)).
      * DVE units (small qtiles) + the first clean 512 cols of qi 8/9:
        Schraudolph bitcast exp on VectorE (y=int32(x*a+b) -> f32 view;
        f32->i32 convert saturates, so masked -30000 scores become
        INT32_MIN = -0.0 and vanish from the sums), then a batched
        tensor_reduce.
  - lse = ln(sums) via exponent/mantissa bitcast split + ACT Ln on the
    mantissa; weighted head-sum chain; host adds the 2 head-group
    partials per batch.
"""

import numpy as np
import ml_dtypes

B, T, E, H = 4, 2048, 1024, 16
D = 64
NCORES = 8
NE = E // 128   # 8 e chunks
PAIRS = 4
NQ = T // 128
BIGNEG = -30000.0
SCH_A = float((1 << 23) / np.log(2.0))
SCH_B = float(127 * (1 << 23) - 486408)
XSPLIT = {8: 512, 9: 512}   # clean cols of these qtiles go to VectorE

_CACHE = {}


def _build():
    import concourse.bass as bass  # noqa: F401
    import concourse.tile as tile
    from concourse import bacc, mybir

    f32 = mybir.dt.float32
    bf16 = mybir.dt.bfloat16
    i32 = mybir.dt.int32
    EXP = mybir.ActivationFunctionType.Exp
    LN = mybir.ActivationFunctionType.Ln
    AX = mybir.AxisListType.X
    MUL = mybir.AluOpType.mult
    ADD = mybir.AluOpType.add
    SHR = mybir.AluOpType.logical_shift_right
    SUB = mybir.AluOpType.subtract
    AND = mybir.AluOpType.bitwise_and
    OR = mybir.AluOpType.bitwise_or
    LN2 = float(np.log(2.0))

    nc = bacc.Bacc("TRN2", target_bir_lowering=False, debug=False,
                   num_devices=NCORES)

    g_ds = [nc.dram_tensor(f"g_q{tq}", [128, NE * 512], bf16,
                           kind="ExternalInput").ap() for tq in range(4)]
    w_ds = [nc.dram_tensor(f"w_p{p}", [128, NE * 256], bf16,
                           kind="ExternalInput").ap() for p in range(4)]
    id_d = nc.dram_tensor("ident", [128, 128], bf16,
                          kind="ExternalInput").ap()
    un_d = nc.dram_tensor("uneg", [128, 128], bf16,
                          kind="ExternalInput").ap()
    wb_d = nc.dram_tensor("wb", [128, 8], f32, kind="ExternalInput").ap()
    out_d = nc.dram_tensor("out_part", [128, 16], f32,
                           kind="ExternalOutput").ap()

    with tile.TileContext(nc) as tc:
        with (
            tc.tile_pool(name="consts", bufs=1) as consts,
            tc.tile_pool(name="big", bufs=1) as big,
            tc.tile_pool(name="qkp", bufs=16) as qkp,
            tc.tile_pool(name="exps", bufs=2) as exps,
            tc.tile_pool(name="schp", bufs=3) as schp,
            tc.tile_pool(name="misc", bufs=1) as misc,
            tc.tile_pool(name="fin", bufs=2) as fin,
            tc.tile_pool(name="ps_lift", bufs=2, space="PSUM") as ps_lift,
            tc.tile_pool(name="ps_big", bufs=1, space="PSUM") as ps_big,
            tc.tile_pool(name="ps_small", bufs=1, space="PSUM") as ps_small,
        ):
            # ---- constants & inputs ---------------------------------------
            ident = consts.tile([128, 128], bf16, name="ident", tag="ident")
            nc.sync.dma_start(out=ident[:], in_=id_d[:])
            uneg = consts.tile([128, 128], bf16, name="uneg", tag="uneg")
            nc.sync.dma_start(out=uneg[:], in_=un_d[:])
            wb = consts.tile([128, 8], f32, name="wb", tag="wb")
            nc.sync.dma_start(out=wb[:], in_=wb_d[:])

            wT, gq = [], []
            wtiles, gtiles = [], []
            for p in range(4):
                t = big.tile([128, NE * 256], bf16, name=f"wT{p}",
                             tag=f"wT{p}")
                wtiles.append(t)
                wT.append(t.rearrange("p (e f) -> p e f", e=NE))
                t = big.tile([128, NE * 512], bf16, name=f"gq{p}",
                             tag=f"gq{p}")
                gtiles.append(t)
                gq.append(t.rearrange("p (e t) -> p e t", e=NE))
            for i in range(4):
                nc.sync.dma_start(out=wtiles[i][:], in_=w_ds[i][:])
                nc.sync.dma_start(out=gtiles[i][:], in_=g_ds[i][:])

            sums = misc.tile([128, 128], f32, name="sums", tag="sums")
            sumsB = misc.tile([128, 128], f32, name="sumsB", tag="sumsB")
            sumsC = misc.tile([128, 128], f32, name="sumsC", tag="sumsC")
            nc.vector.memset(sums[:], 0.0)
            nc.vector.memset(sumsB[:], 0.0)
            nc.vector.memset(sumsC[:], 0.0)

            qk = {}

            def lift_chunk(p, ft, tcq):
                pt = ps_lift.tile([128, 512], f32, name=f"pl{p}{ft}{tcq}",
                                  tag="pslift")
                fo = ft * 128
                for e in range(NE):
                    nc.tensor.matmul(
                        pt[:], lhsT=wT[p][:, e, fo:fo + 128],
                        rhs=gq[tcq][:, e], start=(e == 0), stop=(e == NE - 1))
                nc.vector.tensor_copy(
                    qk[(p, tcq)][:, ft * 512: ft * 512 + 512], pt[:])

            def unit_mms(p, h, qi, pst, k0, k1):
                """Score MMs for keys [k0,k1) of (p,h,qi) into pst cols
                [0, k1-k0). If k1 == kneed the diag mask group is included."""
                kneed = 128 * (qi + 1)
                kf = (kneed - 128) // 512
                lo, hi = 64 * h, 64 * h + 64
                lhsT = qk[(p, qi // 4)][lo:hi,
                                        (qi % 4) * 128: (qi % 4) * 128 + 128]
                mms = []
                stop_full = min(k1, 512 * kf)
                a = k0
                while a < stop_full:
                    c = a // 512
                    mms.append((pst[:, a - k0:a - k0 + 512], lhsT,
                                qk[(p, c)][lo:hi, 512:1024], True, True))
                    a += 512
                dg = []
                if k1 == kneed:
                    a = max(512 * kf, k0)
                    w = kneed - a
                    dg = [(pst[:, kneed - 128 - k0:kneed - k0], ident[:],
                           uneg[:], True, False),
                          (pst[:, a - k0:kneed - k0], lhsT,
                           qk[(p, kf)][lo:hi, 512 + a - 512 * kf:
                                       512 + kneed - 512 * kf],
                           False, True)]
                return mms, dg

            # --- Schraudolph batching: one lane per col//16, batches two
            # consecutive cols (stride 1) of equal width into one reduce.
            sch_lanes = {}
            sch_idx = [0]

            def _lane_flush(lane):
                st = sch_lanes.get(lane)
                if not st:
                    return
                v = st["tile"][:].bitcast(f32)
                slots = st["slots"]  # [(col, width)] ascending cols
                if len(slots) == 2:
                    (cA, wA), (cB, wB) = slots
                    v3 = v.rearrange("p (u x) -> p u x", u=2)
                    out2 = sumsB[:].rearrange(
                        "p (a b) -> p a b", b=1)[:, cA:cA + 2, 0]
                    nc.vector.tensor_reduce(out=out2, in_=v3[:, :, 0:wA],
                                            axis=AX, op=ADD)
                else:
                    (cA, wA), = slots
                    nc.vector.tensor_reduce(
                        out=sumsB[:, cA:cA + 1],
                        in_=v.rearrange("p (u x) -> p u x", u=2)[:, 0, 0:wA],
                        axis=AX, op=ADD)
                del sch_lanes[lane]

            def _sch_flush():
                for lane in list(sch_lanes):
                    _lane_flush(lane)

            def _sch_emit(col, pst, off, width):
                lane = col // 16
                st = sch_lanes.get(lane)
                if st is not None and (len(st["slots"]) == 2
                                       or st["slots"][-1][1] != width
                                       or col != st["slots"][-1][0] + 1):
                    _lane_flush(lane)
                    st = None
                if st is None:
                    sch_idx[0] += 1
                    tl = schp.tile([128, 2048], i32,
                                   name=f"sch{sch_idx[0]}", tag="sch")
                    st = {"tile": tl, "slots": [(col, width)]}
                    sch_lanes[lane] = st
                    slot = 0
                else:
                    st["slots"].append((col, width))
                    slot = 1
                nc.vector.tensor_scalar(
                    out=st["tile"][:, slot * 1024: slot * 1024 + width],
                    in0=pst[:, off:off + width], scalar1=SCH_A,
                    scalar2=SCH_B, op0=MUL, op1=ADD)

            ucnt = [0]

            def emit_group(units):
                """units: list of dicts {p,h,qi,kind,big} emitted together.
                kind: 'act' full range ACT(+xsp sch); 'dve' full range all
                sch; 'h0' keys [0:1024) sch only; 'h1' keys [1024:kneed)
                ACT -> sumsC."""
                tiles, ranges = [], []
                for u in units:
                    kneed = 128 * (u["qi"] + 1)
                    k0, k1 = 0, kneed
                    if u["kind"] == "h0":
                        k1 = 1024
                    elif u["kind"] == "h1":
                        k0 = 1024
                    ranges.append((k0, k1))
                    ucnt[0] += 1
                    if u["big"]:
                        t = ps_big.tile([128, 2048], f32,
                                        name=f"pb{ucnt[0]}", tag="psbig")
                    else:
                        t = ps_small.tile([128, 1024], f32,
                                          name=f"pc{ucnt[0]}", tag="pssmall")
                    tiles.append(t)
                ms = []
                for u, t, (k0, k1) in zip(units, tiles, ranges):
                    ms.append(unit_mms(u["p"], u["h"], u["qi"], t, k0, k1))
                n = max(len(m[0]) for m in ms)
                for i in range(n):
                    for m in ms:
                        if i < len(m[0]):
                            o, l, r, st, sp = m[0][i]
                            nc.tensor.matmul(o, lhsT=l, rhs=r, start=st,
                                             stop=sp)
                for m in ms:
                    for (o, l, r, st, sp) in m[1]:
                        nc.tensor.matmul(o, lhsT=l, rhs=r, start=st, stop=sp)
                for u, pst, (k0, k1) in zip(units, tiles, ranges):
                    kneed = 128 * (u["qi"] + 1)
                    col = (2 * u["p"] + u["h"]) * 16 + u["qi"]
                    kind = u["kind"]
                    if kind in ("dve", "h0"):
                        _sch_emit(col, pst, 0, k1 - k0)
                        continue
                    xsp = XSPLIT.get(u["qi"], 0) if kind == "act" else 0
                    if xsp:
                        _sch_emit(col, pst, 0, xsp)
                    eb = exps.tile([128, 2048], bf16, name=f"eb{ucnt[0]}_{col}",
                                   tag="eb")
                    dst = sums if kind == "act" else sumsC
                    nc.scalar.activation(
                        eb[:, 0:k1 - k0 - xsp], pst[:, k0 - k0 + xsp:k1 - k0],
                        EXP, accum_out=dst[:, col:col + 1])

            # ---- schedule -------------------------------------------------
            for p in range(4):
                for tcq in range(4):
                    qk[(p, tcq)] = qkp.tile([128, 1024], bf16,
                                            name=f"qk{p}{tcq}", tag="qk")

            lift_q = {p: [(p, ft, tcq) for tcq in range(4) for ft in range(2)]
                      for p in range(4)}

            def pop_lift(p, n=1):
                for _ in range(n):
                    if lift_q[p]:
                        lift_chunk(*lift_q[p].pop(0))

            def U(p, h, qi, kind, big):
                return {"p": p, "h": h, "qi": qi, "kind": kind, "big": big}

            # prologue: pair 0; A-smalls on ACT, B-smalls on DVE
            pop_lift(0, 2)
            for i in range(8):
                emit_group([U(0, 0, i, "act", False),
                            U(0, 1, i, "dve", True)])
                if i < 6:
                    pop_lift(0, 1)
                if i >= 4:
                    pop_lift(1, 1)

            # stages 0..2: bigs(s) paired with DVE smalls(s+1)
            for s in range(3):
                for i in range(8):
                    emit_group([U(s, 0, 8 + i, "act", True),
                                U(s + 1, 1, i, "dve", False)])
                    if i < 4:
                        pop_lift(s + 1, 1)
                    elif s < 2:
                        pop_lift(s + 2, 1)
                    else:
                        pop_lift(3, 1)
                    emit_group([U(s, 1, 8 + i, "act", True),
                                U(s + 1, 0, i, "dve", False)])
                    if s < 2 and i >= 4:
                        pop_lift(s + 2, 1)
            # stage 3: A-bigs full on ACT; B-bigs split into h0 (alternating
            # DVE/ACT... h0 kind: even i -> DVE sch, odd i -> dve too keeps
            # DVE loaded while ACT takes h1) + h1 on ACT -> sumsC
            for i in range(8):
                emit_group([U(3, 0, 8 + i, "act", True),
                            U(3, 1, 8 + i, "h0", False)])
                emit_group([U(3, 1, 8 + i, "h1", False)])
            _sch_flush()

            # ---- finale ---------------------------------------------------
            tot = fin.tile([128, 128], f32, name="tot", tag="tot")
            nc.vector.tensor_tensor(out=tot[:], in0=sums[:], in1=sumsB[:],
                                    op=ADD)
            nc.vector.tensor_tensor(out=tot[:], in0=tot[:], in1=sumsC[:],
                                    op=ADD)
            u = tot[:].bitcast(i32)
            ei = fin.tile([128, 128], i32, name="ei", tag="ei")
            nc.vector.tensor_scalar(out=ei[:], in0=u, scalar1=23,
                                    scalar2=None, op0=SHR)
            ef = fin.tile([128, 128], f32, name="ef", tag="ef")
            nc.vector.tensor_copy(ef[:], ei[:])
            nc.vector.tensor_scalar(out=ef[:], in0=ef[:], scalar1=127.0,
                                    scalar2=None, op0=SUB)
            mb = fin.tile([128, 128], i32, name="mb", tag="mb")
            nc.vector.tensor_scalar(out=mb[:], in0=u, scalar1=0x007FFFFF,
                                    scalar2=0x3F800000, op0=AND, op1=OR)
            lnm = fin.tile([128, 128], f32, name="lnm", tag="lnm")
            nc.scalar.activation(lnm[:], mb[:].bitcast(f32), LN)
            lse = fin.tile([128, 128], f32, name="lse", tag="lse")
            nc.vector.scalar_tensor_tensor(
                out=lse[:], in0=ef[:], scalar=LN2, in1=lnm[:],
                op0=MUL, op1=ADD)
            acc = [fin.tile([128, 16], f32, name=f"acc{i}", tag=f"acc{i}")
                   for i in range(2)]
            nc.vector.memset(acc[0][:], 0.0)
            cur = 0
            for lh in range(8):
                nxt = 1 - cur
                nc.vector.scalar_tensor_tensor(
                    out=acc[nxt][:], in0=lse[:, lh * 16:lh * 16 + 16],
                    scalar=wb[:, lh:lh + 1], in1=acc[cur][:],
                    op0=MUL, op1=ADD)
                cur = nxt
            nc.sync.dma_start(out=out_d[:], in_=acc[cur][:])

    nc.compile()
    return nc


def _get_nc():
    if "nc" not in _CACHE:
        _CACHE["nc"] = _build()
    return _CACHE["nc"]


def kernel(g, W_lift, W_proj):
    from concourse.bass_utils import run_bass_kernel_spmd

    bf = ml_dtypes.bfloat16
    g = np.asarray(g, dtype=np.float32)
    W_lift = np.asarray(W_lift, dtype=np.float32)
    W_proj = np.asarray(W_proj, dtype=np.float32)

    nc = _get_nc()
    ident = np.eye(128, dtype=np.float32).astype(bf)
    uneg = (np.triu(np.full((128, 128), BIGNEG, dtype=np.float32), 1)
            ).astype(bf)
    w_all = W_proj.sum(axis=0).astype(np.float32)

    in_maps = []
    for core in range(NCORES):
        b, hg = core // 2, core % 2
        gt = g[b].T.reshape(NE, 128, 4, 512).transpose(1, 2, 0, 3)
        in_map = {"ident": ident, "uneg": uneg}
        for tq in range(4):
            in_map[f"g_q{tq}"] = np.ascontiguousarray(gt[:, tq]).reshape(
                128, NE * 512).astype(bf)
        for p in range(PAIRS):
            h0 = hg * 8 + 2 * p
            h1 = h0 + 1
            rows = (list(range(h0 * D, h0 * D + D))
                    + list(range(h1 * D, h1 * D + D))
                    + list(range(E + h0 * D, E + h0 * D + D))
                    + list(range(E + h1 * D, E + h1 * D + D)))
            w_slice = W_lift[rows, :]
            wp = np.ascontiguousarray(
                w_slice.T.reshape(NE, 128, 256).transpose(1, 0, 2)
            ).reshape(128, NE * 256).astype(bf)
            in_map[f"w_p{p}"] = wp
        wbv = np.broadcast_to(w_all[hg * 8: hg * 8 + 8],
                              (128, 8)).astype(np.float32)
        in_map["wb"] = np.ascontiguousarray(wbv)
        in_maps.append(in_map)

    res = run_bass_kernel_spmd(nc, in_maps, core_ids=list(range(NCORES)))
    _CACHE["last_results"] = res
    _CACHE["last_in_maps"] = in_maps

    out = np.zeros((B, T), dtype=np.float32)
    for core in range(NCORES):
        b = core // 2
        part = res.results[core]["out_part"]
        out[b] += part.T.reshape(-1)
    return out


# revision 12
# speedup vs baseline: 1.2146x; 1.2146x over previous
"""Trainium2 Bass kernel for nn_CausalSelfAttentionSynapse.

Math (per reference):
    qk = g @ W_lift.T                       # (B,T,2E)
    q,k heads of dim D=64; scores = q@k.T causal-masked
    lse[b,h,t] = logsumexp_{j<=t} scores[b,h,t,j]
    out[b,t]  = sum_h lse[b,h,t] * w[h],  w[h] = sum_g W_proj[g,h]

Sharding: 8 cores = 4 batches x 2 head-groups (8 heads each).

Per-core design (v5):
  - Host pre-transposes g[b] / W_lift head-group rows into e-major bf16.
    DMAs are merged into few prioritized transfers (SP HWDGE ring is
    FIFO): [consts|w_p0|g_q0] first so lift starts ~6us in.
  - Lift (bf16, e on partitions) lands qkT per pair in score layout:
    partitions 0-63 head A d, 64-127 head B d; per t-quarter tile
    cols 0:512 q, 512:1024 k.
  - Scores are emitted as pieces <= 1024 keys wide (qi>=8 splits into
    h0 = keys [0,1024) and h1 = [1024,kneed)) on a single 3-deep
    [128,1024] PSUM rotation - uniform rotation keeps the PE from
    ping-ponging on one bank group (HAM throttle death spiral).
  - Causal mask of each diagonal block = identity x strict-upper
    (-30000) matmul; within a PSUM bank all writers form one
    contiguous accumulation group.
  - exp+row-sum pieces are statically split between ScalarE (Exp with
    fused accum_out) and VectorE (Schraudolph bitcast exp: int32
    (x*a+b) viewed as f32; the f32->i32 convert saturates so masked
    scores become -0.0) + batched tensor_reduce. Three accumulator
    tiles (sums/sumsB/sumsC) keep writer columns disjoint.
  - lse = ln(sums) via exponent/mantissa bitcast split + ACT Ln;
    weighted head-sum chain; host adds the 2 head-group partials.
"""

import numpy as np
import ml_dtypes

B, T, E, H = 4, 2048, 1024, 16
D = 64
NCORES = 8
NE = E // 128
PAIRS = 4
NQ = T // 128
BIGNEG = -30000.0
SCH_A = float((1 << 23) / np.log(2.0))
SCH_B = float(127 * (1 << 23) - 486408)

_CACHE = {}


def _eng(kind, h, qi):
    """Static engine assignment for exp+sum pieces."""
    if kind == "s":
        return "dve" if qi < 4 else "act"
    if kind == "h1":
        return "dve" if qi < 12 else "act"
    # h0
    return "dve" if qi <= 10 else "act"


def _build():
    import concourse.bass as bass  # noqa: F401
    import concourse.tile as tile
    from concourse import bacc, mybir

    f32 = mybir.dt.float32
    bf16 = mybir.dt.bfloat16
    i32 = mybir.dt.int32
    EXP = mybir.ActivationFunctionType.Exp
    LN = mybir.ActivationFunctionType.Ln
    AX = mybir.AxisListType.X
    MUL = mybir.AluOpType.mult
    ADD = mybir.AluOpType.add
    SHR = mybir.AluOpType.logical_shift_right
    SUB = mybir.AluOpType.subtract
    AND = mybir.AluOpType.bitwise_and
    OR = mybir.AluOpType.bitwise_or
    LN2 = float(np.log(2.0))

    nc = bacc.Bacc("TRN2", target_bir_lowering=False, debug=False,
                   num_devices=NCORES)

    # merged, priority-ordered inputs (SP HWDGE ring is FIFO)
    pre_d = nc.dram_tensor("pre", [128, 6400], bf16,
                           kind="ExternalInput").ap()
    gq1_d = nc.dram_tensor("gq1", [128, 4096], bf16,
                           kind="ExternalInput").ap()
    wp1_d = nc.dram_tensor("wp1", [128, 2048], bf16,
                           kind="ExternalInput").ap()
    gq23_d = nc.dram_tensor("gq23", [128, 8192], bf16,
                            kind="ExternalInput").ap()
    wp23_d = nc.dram_tensor("wp23", [128, 4096], bf16,
                            kind="ExternalInput").ap()
    wb_d = nc.dram_tensor("wb", [128, 8], f32, kind="ExternalInput").ap()
    out_d = nc.dram_tensor("out_part", [128, 16], f32,
                           kind="ExternalOutput").ap()

    with tile.TileContext(nc) as tc:
        with (
            tc.tile_pool(name="big", bufs=1) as big,
            tc.tile_pool(name="qkp", bufs=16) as qkp,
            tc.tile_pool(name="exps", bufs=2) as exps,
            tc.tile_pool(name="schp", bufs=3) as schp,
            tc.tile_pool(name="misc", bufs=1) as misc,
            tc.tile_pool(name="fin", bufs=2) as fin,
            tc.tile_pool(name="ps_lift", bufs=2, space="PSUM") as ps_lift,
            tc.tile_pool(name="ps_sc", bufs=3, space="PSUM") as ps_sc,
        ):
            # ---- inputs (merged tiles, priority order) --------------------
            pre = big.tile([128, 6400], bf16, name="pre", tag="pre")
            nc.sync.dma_start(out=pre[:], in_=pre_d[:])
            gq1 = big.tile([128, 4096], bf16, name="gq1", tag="gq1")
            nc.sync.dma_start(out=gq1[:], in_=gq1_d[:])
            wp1 = big.tile([128, 2048], bf16, name="wp1", tag="wp1")
            nc.sync.dma_start(out=wp1[:], in_=wp1_d[:])
            gq23 = big.tile([128, 8192], bf16, name="gq23", tag="gq23")
            nc.sync.dma_start(out=gq23[:], in_=gq23_d[:])
            wp23 = big.tile([128, 4096], bf16, name="wp23", tag="wp23")
            nc.sync.dma_start(out=wp23[:], in_=wp23_d[:])
            wb = misc.tile([128, 8], f32, name="wb", tag="wb")
            nc.sync.dma_start(out=wb[:], in_=wb_d[:])

            ident = pre[:, 0:128]
            uneg = pre[:, 128:256]
            wT = [pre[:, 256:2304].rearrange("p (e f) -> p e f", e=NE),
                  wp1[:].rearrange("p (e f) -> p e f", e=NE),
                  wp23[:, 0:2048].rearrange("p (e f) -> p e f", e=NE),
                  wp23[:, 2048:4096].rearrange("p (e f) -> p e f", e=NE)]
            gq = [pre[:, 2304:6400].rearrange("p (e t) -> p e t", e=NE),
                  gq1[:].rearrange("p (e t) -> p e t", e=NE),
                  gq23[:, 0:4096].rearrange("p (e t) -> p e t", e=NE),
                  gq23[:, 4096:8192].rearrange("p (e t) -> p e t", e=NE)]

            sums = misc.tile([128, 128], f32, name="sums", tag="sums")
            sumsB = misc.tile([128, 128], f32, name="sumsB", tag="sumsB")
            sumsC = misc.tile([128, 128], f32, name="sumsC", tag="sumsC")
            nc.vector.memset(sums[:], 0.0)
            nc.vector.memset(sumsB[:], 0.0)
            nc.vector.memset(sumsC[:], 0.0)

            qk = {}
            for p in range(4):
                for tcq in range(4):
                    qk[(p, tcq)] = qkp.tile([128, 1024], bf16,
                                            name=f"qk{p}{tcq}", tag="qk")

            def lift_chunk(p, ft, tcq):
                pt = ps_lift.tile([128, 512], f32, name=f"pl{p}{ft}{tcq}",
                                  tag="pslift")
                fo = ft * 128
                for e in range(NE):
                    nc.tensor.matmul(
                        pt[:], lhsT=wT[p][:, e, fo:fo + 128],
                        rhs=gq[tcq][:, e], start=(e == 0), stop=(e == NE - 1))
                nc.vector.tensor_copy(
                    qk[(p, tcq)][:, ft * 512: ft * 512 + 512], pt[:])

            def unit_mms(p, h, qi, pst, k0, k1):
                kneed = 128 * (qi + 1)
                kf = (kneed - 128) // 512
                lo, hi = 64 * h, 64 * h + 64
                lhsT = qk[(p, qi // 4)][lo:hi,
                                        (qi % 4) * 128: (qi % 4) * 128 + 128]
                mms = []
                stop_full = min(k1, 512 * kf)
                a = k0
                while a < stop_full:
                    mms.append((pst[:, a - k0:a - k0 + 512], lhsT,
                                qk[(p, a // 512)][lo:hi, 512:1024],
                                True, True))
                    a += 512
                dg = []
                if k1 == kneed:
                    a = max(512 * kf, k0)
                    dg = [(pst[:, kneed - 128 - k0:kneed - k0], ident,
                           uneg, True, False),
                          (pst[:, a - k0:kneed - k0], lhsT,
                           qk[(p, kf)][lo:hi, 512 + a - 512 * kf:
                                       512 + kneed - 512 * kf],
                           False, True)]
                return mms, dg

            # --- Schraudolph batching: up to 2 pieces, ascending cols with
            # stride 1 or 16, same width and same destination tile.
            sch = {"tile": None, "slots": [], "dest": None, "idx": 0}

            def _sch_flush():
                if sch["tile"] is None:
                    return
                v = sch["tile"][:].bitcast(f32)
                slots = sch["slots"]
                dest = sch["dest"]
                if len(slots) == 2:
                    (cA, wA), (cB, _) = slots
                    stp = cB - cA
                    v3 = v.rearrange("p (u x) -> p u x", u=2)
                    if stp == 16:
                        ov = dest[:].rearrange("p (a b) -> p a b", b=16)
                        out2 = ov[:, cA // 16: cA // 16 + 2, cA % 16]
                    else:
                        out2 = dest[:].rearrange(
                            "p (a b) -> p a b", b=1)[:, cA:cA + 2, 0]
                    nc.vector.tensor_reduce(out=out2, in_=v3[:, :, 0:wA],
                                            axis=AX, op=ADD)
                else:
                    (cA, wA), = slots
                    nc.vector.tensor_reduce(
                        out=dest[:, cA:cA + 1],
                        in_=v.rearrange("p (u x) -> p u x", u=2)[:, 0, 0:wA],
                        axis=AX, op=ADD)
                sch["tile"] = None
                sch["slots"] = []
                sch["dest"] = None

            def _sch_emit(col, pst, off, width, dest):
                if sch["tile"] is not None:
                    pc, pw = sch["slots"][-1]
                    if (len(sch["slots"]) == 2 or pw != width
                            or sch["dest"] is not dest
                            or col - sch["slots"][0][0] not in (1, 16)):
                        _sch_flush()
                if sch["tile"] is None:
                    sch["idx"] += 1
                    sch["tile"] = schp.tile([128, 2048], i32,
                                            name=f"sch{sch['idx']}",
                                            tag="sch")
                    sch["slots"] = [(col, width)]
                    sch["dest"] = dest
                    slot = 0
                else:
                    sch["slots"].append((col, width))
                    slot = 1
                nc.vector.tensor_scalar(
                    out=sch["tile"][:, slot * 1024: slot * 1024 + width],
                    in0=pst[:, off:off + width], scalar1=SCH_A,
                    scalar2=SCH_B, op0=MUL, op1=ADD)

            ucnt = [0]

            def emit_group(units):
                """units: list of (p, h, qi, kind); kind in s/h0/h1."""
                tiles, ranges = [], []
                for (p, h, qi, kind) in units:
                    kneed = 128 * (qi + 1)
                    k0 = 1024 if kind == "h1" else 0
                    k1 = 1024 if kind == "h0" else kneed
                    ranges.append((k0, k1))
                    ucnt[0] += 1
                    tiles.append(ps_sc.tile([128, 1024], f32,
                                            name=f"sc{ucnt[0]}", tag="psc"))
                ms = [unit_mms(p, h, qi, t, k0, k1)
                      for (p, h, qi, kind), t, (k0, k1)
                      in zip(units, tiles, ranges)]
                n = max(len(m[0]) for m in ms)
                for i in range(n):
                    for m in ms:
                        if i < len(m[0]):
                            o, l, r, st, sp = m[0][i]
                            nc.tensor.matmul(o, lhsT=l, rhs=r, start=st,
                                             stop=sp)
                for m in ms:
                    for (o, l, r, st, sp) in m[1]:
                        nc.tensor.matmul(o, lhsT=l, rhs=r, start=st, stop=sp)
                for (p, h, qi, kind), pst, (k0, k1) in zip(units, tiles,
                                                           ranges):
                    col = (2 * p + h) * 16 + qi
                    w = k1 - k0
                    if kind == "h1":
                        dst_act, dst_dve = sumsC, sumsC
                    elif kind == "h0":
                        dst_act, dst_dve = sums, sumsB
                    else:
                        dst_act, dst_dve = sums, sumsB
                    if _eng(kind, h, qi) == "dve":
                        _sch_emit(col, pst, 0, w, dst_dve)
                        continue
                    eb = exps.tile([128, 1024], bf16, name=f"eb{ucnt[0]}_{col}",
                                   tag="eb")
                    nc.scalar.activation(eb[:, 0:w], pst[:, 0:w], EXP,
                                         accum_out=dst_act[:, col:col + 1])

            # ---- schedule -------------------------------------------------
            lift_q = {p: [(p, ft, tcq) for tcq in range(4) for ft in range(2)]
                      for p in range(4)}

            def pop_lift(p, n=1):
                for _ in range(n):
                    if lift_q[p]:
                        lift_chunk(*lift_q[p].pop(0))

            # prologue: pair 0 smalls
            pop_lift(0, 2)
            for i in range(8):
                emit_group([(0, 0, i, "s"), (0, 1, i, "s")])
                if i < 6:
                    pop_lift(0, 1)
                if i >= 4:
                    pop_lift(1, 1)

            # stages 0..2: pair-s deep pieces + pair-(s+1) smalls
            for s in range(3):
                for i in range(8):
                    emit_group([(s, 0, 8 + i, "h0"), (s, 1, 8 + i, "h0")])
                    if i < 4:
                        pop_lift(s + 1, 1)
                    elif s < 2:
                        pop_lift(s + 2, 1)
                    emit_group([(s + 1, 0, i, "s"), (s + 1, 1, i, "s")])
                    emit_group([(s, 0, 8 + i, "h1"), (s, 1, 8 + i, "h1")])
                    if s < 2 and i >= 4:
                        pop_lift(s + 2, 1)
            # stage 3: pair-3 deep pieces
            for i in range(8):
                emit_group([(3, 0, 8 + i, "h0"), (3, 1, 8 + i, "h0")])
                emit_group([(3, 0, 8 + i, "h1"), (3, 1, 8 + i, "h1")])
            _sch_flush()

            # ---- finale ---------------------------------------------------
            tot = fin.tile([128, 128], f32, name="tot", tag="tot")
            nc.vector.tensor_tensor(out=tot[:], in0=sums[:], in1=sumsB[:],
                                    op=ADD)
            nc.vector.tensor_tensor(out=tot[:], in0=tot[:], in1=sumsC[:],
                                    op=ADD)
            u = tot[:].bitcast(i32)
            ei = fin.tile([128, 128], i32, name="ei", tag="ei")
            nc.vector.tensor_scalar(out=ei[:], in0=u, scalar1=23,
                                    scalar2=None, op0=SHR)
            ef = fin.tile([128, 128], f32, name="ef", tag="ef")
            nc.vector.tensor_copy(ef[:], ei[:])
            nc.vector.tensor_scalar(out=ef[:], in0=ef[:], scalar1=127.0,
                                    scalar2=None, op0=SUB)
            mb = fin.tile([128, 128], i32, name="mb", tag="mb")
            nc.vector.tensor_scalar(out=mb[:], in0=u, scalar1=0x007FFFFF,
                                    scalar2=0x3F800000, op0=AND, op1=OR)
            lnm = fin.tile([128, 128], f32, name="lnm", tag="lnm")
            nc.scalar.activation(lnm[:], mb[:].bitcast(f32), LN)
            lse = fin.tile([128, 128], f32, name="lse", tag="lse")
            nc.vector.scalar_tensor_tensor(
                out=lse[:], in0=ef[:], scalar=LN2, in1=lnm[:],
                op0=MUL, op1=ADD)
            acc = [fin.tile([128, 16], f32, name=f"acc{i}", tag=f"acc{i}")
                   for i in range(2)]
            nc.vector.memset(acc[0][:], 0.0)
            cur = 0
            for lh in range(8):
                nxt = 1 - cur
                nc.vector.scalar_tensor_tensor(
                    out=acc[nxt][:], in0=lse[:, lh * 16:lh * 16 + 16],
                    scalar=wb[:, lh:lh + 1], in1=acc[cur][:],
                    op0=MUL, op1=ADD)
                cur = nxt
            nc.sync.dma_start(out=out_d[:], in_=acc[cur][:])

    nc.compile()
    return nc


def _get_nc():
    if "nc" not in _CACHE:
        _CACHE["nc"] = _build()
    return _CACHE["nc"]


def kernel(g, W_lift, W_proj):
    from concourse.bass_utils import run_bass_kernel_spmd

    bf = ml_dtypes.bfloat16
    g = np.asarray(g, dtype=np.float32)
    W_lift = np.asarray(W_lift, dtype=np.float32)
    W_proj = np.asarray(W_proj, dtype=np.float32)

    nc = _get_nc()
    ident = np.eye(128, dtype=np.float32).astype(bf)
    uneg = (np.triu(np.full((128, 128), BIGNEG, dtype=np.float32), 1)
            ).astype(bf)
    w_all = W_proj.sum(axis=0).astype(np.float32)

    in_maps = []
    for core in range(NCORES):
        b, hg = core // 2, core % 2
        # g quarters: [128, e(8), 512] each
        gt = g[b].T.reshape(NE, 128, 4, 512).transpose(1, 2, 0, 3)
        gqs = [np.ascontiguousarray(gt[:, tq]).reshape(128, NE * 512)
               .astype(bf) for tq in range(4)]
        wps = []
        for p in range(PAIRS):
            h0 = hg * 8 + 2 * p
            h1 = h0 + 1
            rows = (list(range(h0 * D, h0 * D + D))
                    + list(range(h1 * D, h1 * D + D))
                    + list(range(E + h0 * D, E + h0 * D + D))
                    + list(range(E + h1 * D, E + h1 * D + D)))
            w_slice = W_lift[rows, :]
            wps.append(np.ascontiguousarray(
                w_slice.T.reshape(NE, 128, 256).transpose(1, 0, 2)
            ).reshape(128, NE * 256).astype(bf))
        pre = np.concatenate([ident, uneg, wps[0], gqs[0]], axis=1)
        wbv = np.broadcast_to(w_all[hg * 8: hg * 8 + 8],
                              (128, 8)).astype(np.float32)
        in_maps.append({
            "pre": np.ascontiguousarray(pre),
            "gq1": gqs[1],
            "wp1": wps[1],
            "gq23": np.ascontiguousarray(
                np.concatenate([gqs[2], gqs[3]], axis=1)),
            "wp23": np.ascontiguousarray(
                np.concatenate([wps[2], wps[3]], axis=1)),
            "wb": np.ascontiguousarray(wbv),
        })

    res = run_bass_kernel_spmd(nc, in_maps, core_ids=list(range(NCORES)))
    _CACHE["last_results"] = res
    _CACHE["last_in_maps"] = in_maps

    out = np.zeros((B, T), dtype=np.float32)
    for core in range(NCORES):
        b = core // 2
        part = res.results[core]["out_part"]
        out[b] += part.T.reshape(-1)
    return out
